# revision 6
# baseline (speedup 1.0000x reference)
"""Criss-cross attention (CCAttention) Trainium2 kernel.

Shapes (hardcoded): x [8, 288, 128, 128] f32, Wq/Wk [36, 288], Wv [288, 288],
bq/bk [36], bv [288], eca_w [3], gamma [1]. Output [8, 288, 128, 128] f32.

Sharding: pure data parallel - one batch element per NeuronCore (8 cores).

Device computes att = gamma*(outH+outW) in [w, h, c] layout; the host adds
the ECA/residual term fac[c]*x (fac = 1 + gamma*sigmoid(eca(mean x))) during
the output transpose. The joint softmax uses unnormalized branch sums:
U = sum exp(s) V and Z carried as an extra V column holding 1/gamma, so the
final scale gamma/Z is reciprocal(Z').

Phase 1 (column branch, per w): q/k/v projections from xw [c+1, w, h],
column scores ST[h',h] = K_w.T Q_w, est = exp(ST)*(1-I), UH|ZH' = est.T @
[VT_w | 1/g]. VT and UH|ZH' staged interleaved to DRAM bf16 as
vu[h, w, {vt,uh}, c] so each phase-2 descriptor moves 1156 contiguous bytes.
Phase 2 (row branch, per h): strided re-reads perform the spatial transpose;
UW|ZW' accumulates onto the loaded UH|ZH' via an identity matmul in the same
PSUM; att = U*recip(Z'). All phase-2 score matmuls+exps are hoisted ahead of
the UW loops so the PE stays busy across the phase-1 -> phase-2 barrier.

Perf structure: a warmup burst un-throttles the PE HAM clock gate (1.2 ->
2.4 GHz) and the dense matmul stream keeps it warm; PSUM drains are split
across DVE and ACT. Small-K matmuls run as concurrent pairs in disjoint PE
row groups (rows 0-63 / 64-127): the score matmuls (K=36) and the
c'=256..289 projection chunk (K=33) each pack two per issue, fed by q/k
copies duplicated at partitions 64-99.
"""

import sys

sys.path.insert(0, "/opt/trn_rl_repo")

import numpy as np
import ml_dtypes

B, C, H, W = 8, 288, 128, 128
CQ = 36
KOFF = 64          # k block starts at column/partition 64 of the packed qk
N_PIX = H * W
BF16 = ml_dtypes.bfloat16

GW = 16  # w-group size in phase 1
GH = 16  # h-group size in phase 2
N_WARM = 24

_CACHE = {}


def _build_nc():
    import concourse.bass as bass
    import concourse.tile as tile
    import concourse.mybir as mybir
    from concourse import bacc

    f32 = mybir.dt.float32
    bf16 = mybir.dt.bfloat16
    AF = mybir.ActivationFunctionType

    nc = bacc.Bacc()

    xw = nc.dram_tensor("xw", [C + 1, W, H], bf16, kind="ExternalInput")
    wqkT = nc.dram_tensor("wqkT", [C + 1, 100], bf16, kind="ExternalInput")
    wvT = nc.dram_tensor("wvT", [C + 1, C], bf16, kind="ExternalInput")
    mask4 = nc.dram_tensor("mask4", [H, 4, H], bf16, kind="ExternalInput")
    identd = nc.dram_tensor("identd", [128, 128], bf16, kind="ExternalInput")
    rgcol = nc.dram_tensor("rgcol", [128, GW], bf16, kind="ExternalInput")
    out = nc.dram_tensor("out", [W, H, C], bf16, kind="ExternalOutput")

    with tile.TileContext(nc) as tc:
        with tc.tile_pool(name="persist", bufs=1) as persist, \
             tc.tile_pool(name="dram", bufs=1, space="DRAM") as dpool:
            # interleaved staging: [h', w, {vt, uh}, c | Z']
            vu_st = dpool.tile([H, W, 2, C + 1], bf16)

            # q/k hold two copies: rows 0:36 and rows 64:100, so score
            # matmuls can run as pairs in disjoint PE row groups.
            q_sb = persist.tile([100, N_PIX], bf16)   # w-major: n = w*128 + h
            k_sb = persist.tile([100, N_PIX], bf16)
            identb = persist.tile([128, 128], bf16)
            nc.gpsimd.dma_start(out=identb[:, :], in_=identd[:, :])
            mask_sb = persist.tile([H, 4, H], bf16)
            nc.gpsimd.dma_start(out=mask_sb[:, :, :], in_=mask4[:, :, :])
            rgcol_sb = persist.tile([128, GW], bf16)
            nc.gpsimd.dma_start(out=rgcol_sb[:, :], in_=rgcol[:, :])
            scratch = persist.tile([128, 512], bf16)
            nc.vector.memset(scratch[:, :], 0.0)

            wqk_sb = []
            wv_sb = []
            for (ofs, cnt) in [(0, 128), (128, 128)]:
                t = persist.tile([cnt, 100], bf16, tag=f"wqk{ofs}")
                nc.gpsimd.dma_start(out=t[:, :], in_=wqkT[ofs:ofs + cnt, :])
                wqk_sb.append(t)
                t = persist.tile([cnt, C], bf16, tag=f"wv{ofs}")
                nc.gpsimd.dma_start(out=t[:, :], in_=wvT[ofs:ofs + cnt, :])
                wv_sb.append(t)
            # chunk-3 weights duplicated at rows 0:33 and 64:97 for pairing
            wqk3 = persist.tile([97, 100], bf16)
            nc.gpsimd.dma_start(out=wqk3[0:33, :], in_=wqkT[256:C + 1, :])
            nc.gpsimd.dma_start(out=wqk3[64:97, :], in_=wqkT[256:C + 1, :])
            wv3 = persist.tile([97, C], bf16)
            nc.gpsimd.dma_start(out=wv3[0:33, :], in_=wvT[256:C + 1, :])
            nc.gpsimd.dma_start(out=wv3[64:97, :], in_=wvT[256:C + 1, :])

            # ---------------- Phase 1: column branch (per w) ----------------
            with tc.tile_pool(name="p1x", bufs=2) as p1x, \
                 tc.tile_pool(name="p1g", bufs=2) as p1g, \
                 tc.tile_pool(name="p1s", bufs=6) as p1s, \
                 tc.tile_pool(name="qkp", bufs=1, space="PSUM") as qkp, \
                 tc.tile_pool(name="vtuh", bufs=4, space="PSUM") as vtuh, \
                 tc.tile_pool(name="stp", bufs=2, space="PSUM") as stp:

                # Warmup burst: dense back-to-back matmuls push the PE HAM
                # clock gate to 8/8 before the real stream begins.
                wps = stp.tile([128, 4, 128], f32, name="st4", tag="st4")
                for i in range(N_WARM):
                    nc.tensor.matmul(
                        wps[:, :, :], scratch[:, 0:128], scratch[:, :],
                        start=True, stop=True, skip_group_check=True,
                    )

                def emit_qk_pair(g, pair, xw_c, xw3b):
                    w0 = g * GW
                    qk_ps = qkp.tile([100, 2, 512], f32, name="qk_ps", tag="qk")
                    for s in range(2):
                        co = (pair * 8 + s * 4) * H
                        for j in range(2):
                            nc.tensor.matmul(
                                qk_ps[:, s, :], wqk_sb[j][:, :],
                                xw_c[j][:, co:co + 512],
                                start=(j == 0), stop=False,
                                skip_group_check=True,
                            )
                    # chunk-3 of both sub-slots as a concurrent row-group pair
                    co0 = (pair * 8) * H
                    nc.tensor.matmul(
                        qk_ps[:, 0, :], wqk3[0:33, :], xw3b[0:33, co0:co0 + 512],
                        start=False, stop=True, skip_group_check=True,
                    )
                    nc.tensor.matmul(
                        qk_ps[:, 1, :], wqk3[64:97, :],
                        xw3b[64:97, co0 + 512:co0 + 1024],
                        start=False, stop=True, skip_group_check=True,
                    )
                    fo = (w0 + pair * 8) * H
                    nc.vector.tensor_copy(
                        out=q_sb[0:CQ, fo:fo + 1024], in_=qk_ps[0:CQ, :, :]
                    )
                    nc.scalar.copy(
                        out=k_sb[0:CQ, fo:fo + 1024],
                        in_=qk_ps[KOFF:KOFF + CQ, :, :]
                    )
                    nc.vector.tensor_copy(
                        out=q_sb[64:64 + CQ, fo:fo + 1024], in_=qk_ps[0:CQ, :, :]
                    )
                    nc.scalar.copy(
                        out=k_sb[64:64 + CQ, fo:fo + 1024],
                        in_=qk_ps[KOFF:KOFF + CQ, :, :]
                    )

                def emit_vt_pair(wp, xw_c, xw3b, vug):
                    # two 1-bank PSUM tiles so the chunk-3 matmuls of the two
                    # w's run as a concurrent row-group pair
                    vta = vtuh.tile([128, 512], f32, name="vt1", tag="vtuh")
                    vtb = vtuh.tile([128, 512], f32, name="vt1", tag="vtuh")
                    coa = (wp * 2) * H
                    cob = (wp * 2 + 1) * H
                    for j in range(2):
                        nc.tensor.matmul(
                            vta[:, 0:C], xw_c[j][:, coa:coa + H], wv_sb[j][:, :],
                            start=(j == 0), stop=False, skip_group_check=True,
                        )
                    for j in range(2):
                        nc.tensor.matmul(
                            vtb[:, 0:C], xw_c[j][:, cob:cob + H], wv_sb[j][:, :],
                            start=(j == 0), stop=False, skip_group_check=True,
                        )
                    nc.tensor.matmul(
                        vta[:, 0:C], xw3b[0:33, coa:coa + H], wv3[0:33, :],
                        start=False, stop=True, skip_group_check=True,
                    )
                    nc.tensor.matmul(
                        vtb[:, 0:C], xw3b[64:97, cob:cob + H], wv3[64:97, :],
                        start=False, stop=True, skip_group_check=True,
                    )
                    nc.vector.tensor_copy(
                        out=vug[:, wp * 2, 0, 0:C], in_=vta[:, 0:C]
                    )
                    nc.scalar.copy(
                        out=vug[:, wp * 2 + 1, 0, 0:C], in_=vtb[:, 0:C]
                    )

                for g in range(W // GW):
                    w0 = g * GW
                    xw_c = []
                    for j, (ofs, cnt) in enumerate([(0, 128), (128, 128)]):
                        t = p1x.tile([cnt, GW * H], bf16, tag=f"xw{j}")
                        nc.sync.dma_start(
                            out=t[:, :], in_=xw[ofs:ofs + cnt, w0:w0 + GW, :]
                        )
                        xw_c.append(t)
                    xw3b = p1x.tile([97, GW * H], bf16, tag="xw3b")
                    nc.sync.dma_start(
                        out=xw3b[0:33, :], in_=xw[256:C + 1, w0:w0 + GW, :]
                    )
                    nc.sync.dma_start(
                        out=xw3b[64:97, :], in_=xw[256:C + 1, w0:w0 + GW, :]
                    )

                    vug = p1g.tile([128, GW, 2, C + 1], bf16, tag="vug")
                    nc.vector.tensor_copy(
                        out=vug[:, :, 0, C:C + 1],
                        in_=rgcol_sb[:, :].rearrange(
                            "p (w a b) -> p w a b", a=1, b=1),
                    )

                    # interleave qk pairs between VT pairs so the qk PSUM
                    # WAR (bufs=1) never stalls the PE
                    emit_qk_pair(g, 0, xw_c, xw3b)
                    emit_vt_pair(0, xw_c, xw3b, vug)
                    emit_vt_pair(1, xw_c, xw3b, vug)
                    emit_qk_pair(g, 1, xw_c, xw3b)
                    for wp in range(2, 8):
                        emit_vt_pair(wp, xw_c, xw3b, vug)

                    # Column scores as row-group pairs (K=36 at rows 0:36 and
                    # 64:100), 4 per PSUM bank; est = exp * mask
                    ests = []   # ests[r][par] covers w = w0+r*8+2*i+par
                    for r in range(2):
                        st4a = stp.tile([128, 4, 128], f32, name="st4", tag="st4")
                        st4b = stp.tile([128, 4, 128], f32, name="st4", tag="st4")
                        for i in range(4):
                            foa = (w0 + r * 8 + 2 * i) * H
                            fob = (w0 + r * 8 + 2 * i + 1) * H
                            nc.tensor.matmul(
                                st4a[:, i, :], k_sb[0:CQ, foa:foa + H],
                                q_sb[0:CQ, foa:foa + H],
                                start=True, stop=True,
                            )
                            nc.tensor.matmul(
                                st4b[:, i, :], k_sb[64:64 + CQ, fob:fob + H],
                                q_sb[64:64 + CQ, fob:fob + H],
                                start=True, stop=True,
                            )
                        esta = p1s.tile([128, 4, 128], bf16, tag="est4")
                        estb = p1s.tile([128, 4, 128], bf16, tag="est4")
                        nc.scalar.activation(esta[:, :, :], st4a[:, :, :], AF.Exp)
                        nc.gpsimd.tensor_mul(
                            out=esta[:, :, :], in0=esta[:, :, :],
                            in1=mask_sb[:, :, :]
                        )
                        nc.scalar.activation(estb[:, :, :], st4b[:, :, :], AF.Exp)
                        nc.gpsimd.tensor_mul(
                            out=estb[:, :, :], in0=estb[:, :, :],
                            in1=mask_sb[:, :, :]
                        )
                        ests.append((esta, estb))

                    # AV: UH_w [h, c | ZH'] = est.T @ [VT_w | 1/g]
                    for wi in range(GW):
                        uh1 = vtuh.tile([128, 512], f32, name="vt1", tag="vtuh")
                        est = ests[wi // 8][wi % 2]
                        nc.tensor.matmul(
                            uh1[:, 0:C + 1],
                            est[:, (wi % 8) // 2, :], vug[:, wi, 0, :],
                            start=True, stop=True,
                        )
                        if wi % 2 == 0:
                            nc.vector.tensor_copy(
                                out=vug[:, wi, 1, :], in_=uh1[:, 0:C + 1]
                            )
                        else:
                            nc.scalar.copy(
                                out=vug[:, wi, 1, :], in_=uh1[:, 0:C + 1]
                            )

                    nc.gpsimd.dma_start(
                        out=vu_st[:, w0:w0 + GW, :, :], in_=vug[:, :, :, :]
                    )

            # ---------------- Phase 2: row branch + combine (per h) ---------
            q_v = q_sb[0:CQ, :].rearrange("p (w h) -> p h w", h=H)
            k_v = k_sb[0:CQ, :].rearrange("p (w h) -> p h w", h=H)
            q2_v = q_sb[64:64 + CQ, :].rearrange("p (w h) -> p h w", h=H)
            k2_v = k_sb[64:64 + CQ, :].rearrange("p (w h) -> p h w", h=H)
            with tc.tile_pool(name="p2b", bufs=3) as p2b, \
                 tc.tile_pool(name="p2s", bufs=33) as p2s, \
                 tc.tile_pool(name="p2r", bufs=8) as p2r, \
                 tc.tile_pool(name="stp2", bufs=2, space="PSUM") as stp2, \
                 tc.tile_pool(name="uwp", bufs=3, space="PSUM") as uwp:
                # All row-score matmuls+exps up front: dense PE work that
                # bridges the phase-1 -> phase-2 staging barrier.
                estws = []
                for g in range(H // GH):
                    h0 = g * GH
                    per_g = []
                    for r in range(2):
                        st4a = stp2.tile([128, 4, 128], f32, name="st4b", tag="st4b")
                        st4b = stp2.tile([128, 4, 128], f32, name="st4b", tag="st4b")
                        for i in range(4):
                            ha = h0 + r * 8 + 2 * i
                            hb = ha + 1
                            nc.tensor.matmul(
                                st4a[:, i, :], k_v[:, ha, :], q_v[:, ha, :],
                                start=True, stop=True,
                            )
                            nc.tensor.matmul(
                                st4b[:, i, :], k2_v[:, hb, :], q2_v[:, hb, :],
                                start=True, stop=True,
                            )
                        estwa = p2s.tile([128, 4, 128], bf16, tag="estw4")
                        estwb = p2s.tile([128, 4, 128], bf16, tag="estw4")
                        nc.scalar.activation(estwa[:, :, :], st4a[:, :, :], AF.Exp)
                        nc.scalar.activation(estwb[:, :, :], st4b[:, :, :], AF.Exp)
                        per_g.append((estwa, estwb))
                    estws.append(per_g)

                for g in range(H // GH):
                    h0 = g * GH
                    vur = p2b.tile([W, GH, 2, C + 1], bf16, tag="vur")
                    nc.sync.dma_start(
                        out=vur[:, :, :, :],
                        in_=vu_st[h0:h0 + GH, :, :, :].rearrange(
                            "h w u c -> w h u c"),
                    )
                    og = p2b.tile([W, GH, C], bf16, tag="og")

                    # UW|ZW' then accumulate the loaded UH|ZH' via identity
                    for p in range(8):
                        uw2 = uwp.tile([128, 2, 512], f32, name="uw2", tag="uw2")
                        for sub in range(2):
                            hi = p * 2 + sub
                            estw = estws[g][hi // 8][hi % 2]
                            nc.tensor.matmul(
                                uw2[:, sub, 0:C + 1],
                                estw[:, (hi % 8) // 2, :], vur[:, hi, 0, :],
                                start=True, stop=False,
                            )
                            nc.tensor.matmul(
                                uw2[:, sub, 0:C + 1], identb[:, :],
                                vur[:, hi, 1, :],
                                start=False, stop=True,
                            )
                        rz2 = p2r.tile([128, 2], f32, tag="rz2")
                        nc.vector.reciprocal(
                            out=rz2[:, :],
                            in_=uw2[:, :, C:C + 1].rearrange("p a o -> p (a o)"),
                        )
                        nc.vector.tensor_scalar_mul(
                            out=og[:, p * 2, :], in0=uw2[:, 0, 0:C],
                            scalar1=rz2[:, 0:1],
                        )
                        nc.scalar.mul(
                            out=og[:, p * 2 + 1, :], in_=uw2[:, 1, 0:C],
                            mul=rz2[:, 1:2],
                        )

                    nc.gpsimd.dma_start(out=out[:, h0:h0 + GH, :], in_=og[:, :, :])

    nc.compile()
    return nc


def _get_nc():
    if "nc" not in _CACHE:
        _CACHE["nc"] = _build_nc()
    return _CACHE["nc"]


def _prep_inputs(x, Wq, bq, Wk, bk, Wv, bv, gamma):
    wqk = np.zeros((C + 1, 100), np.float32)
    wqk[0:C, 0:CQ] = np.asarray(Wq, np.float32).T
    wqk[C, 0:CQ] = np.asarray(bq, np.float32)
    wqk[0:C, KOFF:KOFF + CQ] = np.asarray(Wk, np.float32).T
    wqk[C, KOFF:KOFF + CQ] = np.asarray(bk, np.float32)
    wqkT = wqk.astype(BF16)
    wvT = np.concatenate([np.asarray(Wv, np.float32).T,
                          np.asarray(bv, np.float32)[None, :]]).astype(BF16)
    dmask = (1.0 - np.eye(H, dtype=np.float32))
    mask4 = np.ascontiguousarray(
        np.broadcast_to(dmask[:, None, :], (H, 4, H))).astype(BF16)
    identd = np.eye(128, dtype=np.float32).astype(BF16)
    rgcol = np.full((128, GW), 1.0 / float(np.asarray(gamma).reshape(-1)[0]),
                    np.float32).astype(BF16)

    ones_plane = np.ones((1, W, H), np.float32)
    in_maps = []
    for b in range(B):
        xb = x[b]                                           # [c, h, w]
        xwv = np.ascontiguousarray(xb.transpose(0, 2, 1))   # [c, w, h]
        xwv = np.concatenate([xwv, ones_plane]).astype(BF16)
        in_maps.append({
            "xw": xwv, "wqkT": wqkT, "wvT": wvT, "mask4": mask4,
            "identd": identd, "rgcol": rgcol,
        })
    return in_maps


def kernel(x, Wq, bq, Wk, bk, Wv, bv, eca_w, gamma, _return_results=False,
           **run_kwargs):
    from concourse.bass_utils import run_bass_kernel_spmd

    x = np.asarray(x, np.float32)
    gamma_v = float(np.asarray(gamma, np.float32).reshape(-1)[0])
    eca = np.asarray(eca_w, np.float32)

    # ECA channel factor on host: fac = 1 + gamma*sigmoid(conv1d(mean x))
    y = x.mean(axis=(2, 3))                      # [b, c]
    yp = np.pad(y, ((0, 0), (1, 1)))
    yc = eca[0] * yp[:, :-2] + eca[1] * yp[:, 1:-1] + eca[2] * yp[:, 2:]
    fac = 1.0 + gamma_v / (1.0 + np.exp(-yc))    # [b, c]

    nc = _get_nc()
    in_maps = _prep_inputs(x, Wq, bq, Wk, bk, Wv, bv, gamma)
    res = run_bass_kernel_spmd(nc, in_maps, core_ids=list(range(B)), **run_kwargs)
    out = np.empty((B, C, H, W), np.float32)
    for b in range(B):
        # device output att = gamma*(outH+outW) in [w, h, c]; add fac*x host-side
        att = res.results[b]["out"].astype(np.float32).transpose(2, 1, 0)
        out[b] = att + fac[b][:, None, None] * x[b]
    if _return_results:
        return out, res
    return out


# revision 9
# speedup vs baseline: 1.1283x; 1.1283x over previous
"""Criss-cross attention (CCAttention) Trainium2 kernel.

Shapes (hardcoded): x [8, 288, 128, 128] f32, Wq/Wk [36, 288], Wv [288, 288],
bq/bk [36], bv [288], eca_w [3], gamma [1]. Output [8, 288, 128, 128] f32.

Sharding: pure data parallel - one batch element per NeuronCore (8 cores).

Device computes att = gamma*(outH+outW) in [w, h, c] layout; the host adds
the ECA/residual term fac[c]*x (fac = 1 + gamma*sigmoid(eca(mean x))) during
the output transpose. The joint softmax uses unnormalized branch sums:
U = sum exp(s) V and Z carried as an extra V column holding 1/gamma, so the
final scale gamma/Z is reciprocal(Z').

Phase 1 (column branch, per w): q/k/v projections from xw [c+1, w, h],
column scores ST[h',h] = K_w.T Q_w, est = exp(ST)*(1-I), UH|ZH' = est.T @
[VT_w | 1/g]. VT and UH|ZH' staged interleaved to DRAM bf16 as
vu[h, w, {vt,uh}, c] so each phase-2 descriptor moves 1156 contiguous bytes.
Phase 2 (row branch, per h): strided re-reads perform the spatial transpose;
UW|ZW' accumulates onto the loaded UH|ZH' via an identity matmul in the same
PSUM; att = U*recip(Z'). All phase-2 score matmuls+exps are hoisted ahead of
the UW loops so the PE stays busy across the phase-1 -> phase-2 barrier.

Perf structure: a warmup burst un-throttles the PE HAM clock gate (1.2 ->
2.4 GHz) and the dense matmul stream keeps it warm; PSUM drains are split
across DVE and ACT. Small-K matmuls run as concurrent pairs in disjoint PE
row groups (rows 0-63 / 64-127): the score matmuls (K=36) and the
c'=256..289 projection chunk (K=33) each pack two per issue, fed by q/k
copies duplicated at partitions 64-99.
"""

import sys

sys.path.insert(0, "/opt/trn_rl_repo")

import numpy as np
import ml_dtypes

B, C, H, W = 8, 288, 128, 128
CQ = 36
KOFF = 64          # k block starts at column/partition 64 of the packed qk
N_PIX = H * W
BF16 = ml_dtypes.bfloat16

GW = 16  # w-group size in phase 1
GH = 16  # h-group size in phase 2
N_WARM = 24

_CACHE = {}


def _build_nc():
    import concourse.bass as bass
    import concourse.tile as tile
    import concourse.mybir as mybir
    from concourse import bacc

    f32 = mybir.dt.float32
    bf16 = mybir.dt.bfloat16
    AF = mybir.ActivationFunctionType

    nc = bacc.Bacc()

    xw = nc.dram_tensor("xw", [C + 1, W, H], bf16, kind="ExternalInput")
    wqkT = nc.dram_tensor("wqkT", [C + 1, 100], bf16, kind="ExternalInput")
    wvT = nc.dram_tensor("wvT", [C + 1, C], bf16, kind="ExternalInput")
    mask4 = nc.dram_tensor("mask4", [H, 4, H], bf16, kind="ExternalInput")
    identd = nc.dram_tensor("identd", [128, 128], bf16, kind="ExternalInput")
    rgcol = nc.dram_tensor("rgcol", [128, GW], bf16, kind="ExternalInput")
    out = nc.dram_tensor("out", [W, H, C], bf16, kind="ExternalOutput")

    with tile.TileContext(nc) as tc:
        with tc.tile_pool(name="persist", bufs=1) as persist, \
             tc.tile_pool(name="dram", bufs=1, space="DRAM") as dpool:
            # interleaved staging: [h', w, {vt, uh}, c | Z']
            vu_st = dpool.tile([H, W, 2, C + 1], bf16)

            # q/k hold two copies: rows 0:36 and rows 64:100, so score
            # matmuls can run as pairs in disjoint PE row groups.
            q_sb = persist.tile([100, N_PIX], bf16)   # w-major: n = w*128 + h
            k_sb = persist.tile([100, N_PIX], bf16)
            identb = persist.tile([128, 128], bf16)
            nc.gpsimd.dma_start(out=identb[:, :], in_=identd[:, :])
            mask_sb = persist.tile([H, 4, H], bf16)
            nc.gpsimd.dma_start(out=mask_sb[:, :, :], in_=mask4[:, :, :])
            rgcol_sb = persist.tile([128, GW], bf16)
            nc.gpsimd.dma_start(out=rgcol_sb[:, :], in_=rgcol[:, :])
            scratch = persist.tile([128, 512], bf16)
            nc.vector.memset(scratch[:, :], 0.0)

            wqk_sb = []
            wv_sb = []
            for (ofs, cnt) in [(0, 128), (128, 128)]:
                t = persist.tile([cnt, 100], bf16, tag=f"wqk{ofs}")
                nc.gpsimd.dma_start(out=t[:, :], in_=wqkT[ofs:ofs + cnt, :])
                wqk_sb.append(t)
                t = persist.tile([cnt, C], bf16, tag=f"wv{ofs}")
                nc.gpsimd.dma_start(out=t[:, :], in_=wvT[ofs:ofs + cnt, :])
                wv_sb.append(t)
            # chunk-3 weights duplicated at rows 0:33 and 64:97 for pairing
            wqk3 = persist.tile([97, 100], bf16)
            nc.gpsimd.dma_start(out=wqk3[0:33, :], in_=wqkT[256:C + 1, :])
            nc.gpsimd.dma_start(out=wqk3[64:97, :], in_=wqkT[256:C + 1, :])
            wv3 = persist.tile([97, C], bf16)
            nc.gpsimd.dma_start(out=wv3[0:33, :], in_=wvT[256:C + 1, :])
            nc.gpsimd.dma_start(out=wv3[64:97, :], in_=wvT[256:C + 1, :])

            # ---------------- Phase 1: column branch (per w) ----------------
            with tc.tile_pool(name="p1x", bufs=2) as p1x, \
                 tc.tile_pool(name="p1g", bufs=2) as p1g, \
                 tc.tile_pool(name="p1s", bufs=6) as p1s, \
                 tc.tile_pool(name="qkp", bufs=1, space="PSUM") as qkp, \
                 tc.tile_pool(name="vtuh", bufs=2, space="PSUM") as vtuh, \
                 tc.tile_pool(name="stp", bufs=2, space="PSUM") as stp:

                # Warmup burst: dense back-to-back matmuls push the PE HAM
                # clock gate to 8/8 before the real stream begins.
                wps = stp.tile([128, 4, 128], f32, name="st4", tag="st4")
                for i in range(N_WARM):
                    nc.tensor.matmul(
                        wps[:, :, :], scratch[:, 0:128], scratch[:, :],
                        start=True, stop=True, skip_group_check=True,
                    )

                def emit_qk_pair(g, pair, xw_c, xw3b):
                    w0 = g * GW
                    qk_ps = qkp.tile([100, 2, 512], f32, name="qk_ps", tag="qk")
                    for s in range(2):
                        co = (pair * 8 + s * 4) * H
                        for j in range(2):
                            nc.tensor.matmul(
                                qk_ps[:, s, :], wqk_sb[j][:, :],
                                xw_c[j][:, co:co + 512],
                                start=(j == 0), stop=False,
                                skip_group_check=True,
                            )
                    # chunk-3 of both sub-slots as a concurrent row-group pair
                    co0 = (pair * 8) * H
                    nc.tensor.matmul(
                        qk_ps[:, 0, :], wqk3[0:33, :], xw3b[0:33, co0:co0 + 512],
                        start=False, stop=True, skip_group_check=True,
                    )
                    nc.tensor.matmul(
                        qk_ps[:, 1, :], wqk3[64:97, :],
                        xw3b[64:97, co0 + 512:co0 + 1024],
                        start=False, stop=True, skip_group_check=True,
                    )
                    fo = (w0 + pair * 8) * H
                    nc.vector.tensor_copy(
                        out=q_sb[0:CQ, fo:fo + 1024], in_=qk_ps[0:CQ, :, :]
                    )
                    nc.scalar.copy(
                        out=k_sb[0:CQ, fo:fo + 1024],
                        in_=qk_ps[KOFF:KOFF + CQ, :, :]
                    )
                    nc.vector.tensor_copy(
                        out=q_sb[64:64 + CQ, fo:fo + 1024], in_=qk_ps[0:CQ, :, :]
                    )
                    nc.scalar.copy(
                        out=k_sb[64:64 + CQ, fo:fo + 1024],
                        in_=qk_ps[KOFF:KOFF + CQ, :, :]
                    )

                def emit_vt_pair(wp, xw_c, xw3b, vug):
                    # 2-slot PSUM tile (2 banks) so the chunk-3 matmuls of the
                    # two w's run as a concurrent row-group pair
                    vt2 = vtuh.tile([128, 2, 512], f32, name="vt2", tag="vtuh")
                    for sub in range(2):
                        co = (wp * 2 + sub) * H
                        for j in range(2):
                            nc.tensor.matmul(
                                vt2[:, sub, 0:C], xw_c[j][:, co:co + H],
                                wv_sb[j][:, :],
                                start=(j == 0), stop=False,
                                skip_group_check=True,
                            )
                    co0 = (wp * 2) * H
                    nc.tensor.matmul(
                        vt2[:, 0, 0:C], xw3b[0:33, co0:co0 + H], wv3[0:33, :],
                        start=False, stop=True, skip_group_check=True,
                    )
                    nc.tensor.matmul(
                        vt2[:, 1, 0:C], xw3b[64:97, co0 + H:co0 + 2 * H],
                        wv3[64:97, :],
                        start=False, stop=True, skip_group_check=True,
                    )
                    if wp % 2 == 0:
                        nc.vector.tensor_copy(
                            out=vug[:, wp * 2:wp * 2 + 2, 0, 0:C],
                            in_=vt2[:, :, 0:C]
                        )
                    else:
                        nc.scalar.copy(
                            out=vug[:, wp * 2:wp * 2 + 2, 0, 0:C],
                            in_=vt2[:, :, 0:C]
                        )

                for g in range(W // GW):
                    w0 = g * GW
                    xw_c = []
                    for j, (ofs, cnt) in enumerate([(0, 128), (128, 128)]):
                        t = p1x.tile([cnt, GW * H], bf16, tag=f"xw{j}")
                        nc.sync.dma_start(
                            out=t[:, :], in_=xw[ofs:ofs + cnt, w0:w0 + GW, :]
                        )
                        xw_c.append(t)
                    xw3b = p1x.tile([97, GW * H], bf16, tag="xw3b")
                    nc.sync.dma_start(
                        out=xw3b[0:33, :], in_=xw[256:C + 1, w0:w0 + GW, :]
                    )
                    nc.sync.dma_start(
                        out=xw3b[64:97, :], in_=xw[256:C + 1, w0:w0 + GW, :]
                    )

                    vug = p1g.tile([128, GW, 2, C + 1], bf16, tag="vug")
                    nc.vector.tensor_copy(
                        out=vug[:, :, 0, C:C + 1],
                        in_=rgcol_sb[:, :].rearrange(
                            "p (w a b) -> p w a b", a=1, b=1),
                    )

                    # interleave qk pairs between VT pairs so the qk PSUM
                    # WAR (bufs=1) never stalls the PE
                    emit_qk_pair(g, 0, xw_c, xw3b)
                    emit_vt_pair(0, xw_c, xw3b, vug)
                    emit_vt_pair(1, xw_c, xw3b, vug)
                    emit_qk_pair(g, 1, xw_c, xw3b)
                    for wp in range(2, 8):
                        emit_vt_pair(wp, xw_c, xw3b, vug)

                    # Column scores as row-group pairs (K=36 at rows 0:36 and
                    # 64:100), 4 per PSUM bank; est = exp * mask
                    ests = []   # ests[r][par] covers w = w0+r*8+2*i+par
                    for r in range(2):
                        st4a = stp.tile([128, 4, 128], f32, name="st4", tag="st4")
                        st4b = stp.tile([128, 4, 128], f32, name="st4", tag="st4")
                        for i in range(4):
                            foa = (w0 + r * 8 + 2 * i) * H
                            fob = (w0 + r * 8 + 2 * i + 1) * H
                            nc.tensor.matmul(
                                st4a[:, i, :], k_sb[0:CQ, foa:foa + H],
                                q_sb[0:CQ, foa:foa + H],
                                start=True, stop=True,
                            )
                            nc.tensor.matmul(
                                st4b[:, i, :], k_sb[64:64 + CQ, fob:fob + H],
                                q_sb[64:64 + CQ, fob:fob + H],
                                start=True, stop=True,
                            )
                        esta = p1s.tile([128, 4, 128], bf16, tag="est4")
                        estb = p1s.tile([128, 4, 128], bf16, tag="est4")
                        nc.scalar.activation(esta[:, :, :], st4a[:, :, :], AF.Exp)
                        nc.gpsimd.tensor_mul(
                            out=esta[:, :, :], in0=esta[:, :, :],
                            in1=mask_sb[:, :, :]
                        )
                        nc.scalar.activation(estb[:, :, :], st4b[:, :, :], AF.Exp)
                        nc.gpsimd.tensor_mul(
                            out=estb[:, :, :], in0=estb[:, :, :],
                            in1=mask_sb[:, :, :]
                        )
                        ests.append((esta, estb))

                    # AV: UH_w [h, c | ZH'] = est.T @ [VT_w | 1/g], 2 per drain
                    for wp in range(8):
                        uh2 = vtuh.tile([128, 2, 512], f32, name="vt2", tag="vtuh")
                        for sub in range(2):
                            wi = wp * 2 + sub
                            est = ests[wi // 8][wi % 2]
                            nc.tensor.matmul(
                                uh2[:, sub, 0:C + 1],
                                est[:, (wi % 8) // 2, :], vug[:, wi, 0, :],
                                start=True, stop=True,
                            )
                        if wp % 2 == 1:
                            nc.vector.tensor_copy(
                                out=vug[:, wp * 2:wp * 2 + 2, 1, :],
                                in_=uh2[:, :, 0:C + 1]
                            )
                        else:
                            nc.scalar.copy(
                                out=vug[:, wp * 2:wp * 2 + 2, 1, :],
                                in_=uh2[:, :, 0:C + 1]
                            )

                    nc.gpsimd.dma_start(
                        out=vu_st[:, w0:w0 + GW, :, :], in_=vug[:, :, :, :]
                    )

            # ---------------- Phase 2: row branch + combine (per h) ---------
            q_v = q_sb[0:CQ, :].rearrange("p (w h) -> p h w", h=H)
            k_v = k_sb[0:CQ, :].rearrange("p (w h) -> p h w", h=H)
            q2_v = q_sb[64:64 + CQ, :].rearrange("p (w h) -> p h w", h=H)
            k2_v = k_sb[64:64 + CQ, :].rearrange("p (w h) -> p h w", h=H)
            with tc.tile_pool(name="p2b", bufs=3) as p2b, \
                 tc.tile_pool(name="p2s", bufs=33) as p2s, \
                 tc.tile_pool(name="p2r", bufs=8) as p2r, \
                 tc.tile_pool(name="stp2", bufs=2, space="PSUM") as stp2, \
                 tc.tile_pool(name="uwp", bufs=3, space="PSUM") as uwp:
                # All row-score matmuls+exps up front: dense PE work that
                # bridges the phase-1 -> phase-2 staging barrier.
                estws = []
                for g in range(H // GH):
                    h0 = g * GH
                    per_g = []
                    for r in range(2):
                        st4a = stp2.tile([128, 4, 128], f32, name="st4b", tag="st4b")
                        st4b = stp2.tile([128, 4, 128], f32, name="st4b", tag="st4b")
                        for i in range(4):
                            ha = h0 + r * 8 + 2 * i
                            hb = ha + 1
                            nc.tensor.matmul(
                                st4a[:, i, :], k_v[:, ha, :], q_v[:, ha, :],
                                start=True, stop=True,
                            )
                            nc.tensor.matmul(
                                st4b[:, i, :], k2_v[:, hb, :], q2_v[:, hb, :],
                                start=True, stop=True,
                            )
                        estwa = p2s.tile([128, 4, 128], bf16, tag="estw4")
                        estwb = p2s.tile([128, 4, 128], bf16, tag="estw4")
                        nc.scalar.activation(estwa[:, :, :], st4a[:, :, :], AF.Exp)
                        nc.scalar.activation(estwb[:, :, :], st4b[:, :, :], AF.Exp)
                        per_g.append((estwa, estwb))
                    estws.append(per_g)

                for g in range(H // GH):
                    h0 = g * GH
                    vur = p2b.tile([W, GH, 2, C + 1], bf16, tag="vur")
                    nc.sync.dma_start(
                        out=vur[:, :, :, :],
                        in_=vu_st[h0:h0 + GH, :, :, :].rearrange(
                            "h w u c -> w h u c"),
                    )
                    og = p2b.tile([W, GH, C], bf16, tag="og")

                    # UW|ZW' then accumulate the loaded UH|ZH' via identity
                    for p in range(8):
                        uw2 = uwp.tile([128, 2, 512], f32, name="uw2", tag="uw2")
                        for sub in range(2):
                            hi = p * 2 + sub
                            estw = estws[g][hi // 8][hi % 2]
                            nc.tensor.matmul(
                                uw2[:, sub, 0:C + 1],
                                estw[:, (hi % 8) // 2, :], vur[:, hi, 0, :],
                                start=True, stop=False,
                            )
                            nc.tensor.matmul(
                                uw2[:, sub, 0:C + 1], identb[:, :],
                                vur[:, hi, 1, :],
                                start=False, stop=True,
                            )
                        rz2 = p2r.tile([128, 2], f32, tag="rz2")
                        nc.vector.reciprocal(
                            out=rz2[:, :],
                            in_=uw2[:, :, C:C + 1].rearrange("p a o -> p (a o)"),
                        )
                        nc.vector.tensor_scalar_mul(
                            out=og[:, p * 2, :], in0=uw2[:, 0, 0:C],
                            scalar1=rz2[:, 0:1],
                        )
                        nc.scalar.mul(
                            out=og[:, p * 2 + 1, :], in_=uw2[:, 1, 0:C],
                            mul=rz2[:, 1:2],
                        )

                    nc.gpsimd.dma_start(out=out[:, h0:h0 + GH, :], in_=og[:, :, :])

    nc.compile()
    return nc


def _get_nc():
    if "nc" not in _CACHE:
        _CACHE["nc"] = _build_nc()
    return _CACHE["nc"]


def _prep_inputs(x, Wq, bq, Wk, bk, Wv, bv, gamma):
    wqk = np.zeros((C + 1, 100), np.float32)
    wqk[0:C, 0:CQ] = np.asarray(Wq, np.float32).T
    wqk[C, 0:CQ] = np.asarray(bq, np.float32)
    wqk[0:C, KOFF:KOFF + CQ] = np.asarray(Wk, np.float32).T
    wqk[C, KOFF:KOFF + CQ] = np.asarray(bk, np.float32)
    wqkT = wqk.astype(BF16)
    wvT = np.concatenate([np.asarray(Wv, np.float32).T,
                          np.asarray(bv, np.float32)[None, :]]).astype(BF16)
    dmask = (1.0 - np.eye(H, dtype=np.float32))
    mask4 = np.ascontiguousarray(
        np.broadcast_to(dmask[:, None, :], (H, 4, H))).astype(BF16)
    identd = np.eye(128, dtype=np.float32).astype(BF16)
    rgcol = np.full((128, GW), 1.0 / float(np.asarray(gamma).reshape(-1)[0]),
                    np.float32).astype(BF16)

    ones_plane = np.ones((1, W, H), np.float32)
    in_maps = []
    for b in range(B):
        xb = x[b]                                           # [c, h, w]
        xwv = np.ascontiguousarray(xb.transpose(0, 2, 1))   # [c, w, h]
        xwv = np.concatenate([xwv, ones_plane]).astype(BF16)
        in_maps.append({
            "xw": xwv, "wqkT": wqkT, "wvT": wvT, "mask4": mask4,
            "identd": identd, "rgcol": rgcol,
        })
    return in_maps


def kernel(x, Wq, bq, Wk, bk, Wv, bv, eca_w, gamma, _return_results=False,
           **run_kwargs):
    from concourse.bass_utils import run_bass_kernel_spmd

    x = np.asarray(x, np.float32)
    gamma_v = float(np.asarray(gamma, np.float32).reshape(-1)[0])
    eca = np.asarray(eca_w, np.float32)

    # ECA channel factor on host: fac = 1 + gamma*sigmoid(conv1d(mean x))
    y = x.mean(axis=(2, 3))                      # [b, c]
    yp = np.pad(y, ((0, 0), (1, 1)))
    yc = eca[0] * yp[:, :-2] + eca[1] * yp[:, 1:-1] + eca[2] * yp[:, 2:]
    fac = 1.0 + gamma_v / (1.0 + np.exp(-yc))    # [b, c]

    nc = _get_nc()
    in_maps = _prep_inputs(x, Wq, bq, Wk, bk, Wv, bv, gamma)
    res = run_bass_kernel_spmd(nc, in_maps, core_ids=list(range(B)), **run_kwargs)
    out = np.empty((B, C, H, W), np.float32)
    for b in range(B):
        # device output att = gamma*(outH+outW) in [w, h, c]; add fac*x host-side
        att = res.results[b]["out"].astype(np.float32).transpose(2, 1, 0)
        out[b] = att + fac[b][:, None, None] * x[b]
    if _return_results:
        return out, res
    return out
